# revision 48
# baseline (speedup 1.0000x reference)
"""Distributed Trainium2 kernel for DeepseekV3-style GQA attention.

Problem (hardcoded): B=1, S=4096, H=2048, NQ=16 q heads, NKV=4 kv heads,
D=128, rotate-half RoPE (theta=10000) over full head dim, causal softmax,
o_proj. 8 NeuronCores, tensor-parallel over heads:

  core c: q heads {2c, 2c+1}, kv head c//2 (replicated across the pair),
  Wq/Wk/Wv column-sharded, attention computed flash-style in bf16 with
  f32 PSUM accumulation, attention output produced transposed [j, s],
  AllGathered in 4 sequence chunks, o_proj row-blocks [128, H] per chunk
  interleaved into the attention loop as PE filler work.

v1 changes vs baseline:
  - input DMAs reordered (xt chunk 0 + wq first) and spread across
    engine queues; Scalar (Act) queue issues no DMAs at all
  - causal mask applied additively in PSUM via a 128-col accumulate
    matmul (ident.T @ maskSL) instead of a DVE multiply after exp
  - ptp bufs=4 decouples Act from PE's pt consumption
  - o_proj matmuls pumped one-per-attention-tile as PE fillers right
    after each chunk's AllToAll lands, instead of a serial tail phase
"""
import os
import sys

sys.path.insert(0, "/opt/trn_rl_repo")

import numpy as np
import ml_dtypes

import concourse.bass as bass
import concourse.bacc as bacc
import concourse.mybir as mybir
import concourse.tile as tile
from concourse.bass_utils import run_bass_kernel_spmd

BF16 = mybir.dt.bfloat16
F32 = mybir.dt.float32
NPBF16 = ml_dtypes.bfloat16

B, S, H = 1, 4096, 2048
NQ, NKV, D = 16, 4, 128
THETA = 10000.0
NCORES = 8
HPC = NQ // NCORES          # q heads per core = 2
SC = 512                    # projection s-chunk
NSC = S // SC               # 8
NKT = S // 128              # 32 k tiles of 128
QS = 512                    # attention q supertile
NQS = S // QS               # 8
CHUNK = 1024                # allgather s-chunk
NCH = S // CHUNK            # 4
SCALE = 1.0 / float(np.sqrt(D))

_cached = {}


def _build():
    nc = bacc.Bacc("TRN2", target_bir_lowering=False, debug=False,
                   num_devices=NCORES)

    xT = nc.declare_dram_parameter("xT", [NSC, 128, 16 * SC], BF16, isOutput=False)
    wq = nc.declare_dram_parameter("wq", [128, 16 * HPC * D], BF16, isOutput=False)
    wk = nc.declare_dram_parameter("wk", [128, 16 * D], BF16, isOutput=False)
    wv = nc.declare_dram_parameter("wv", [128, 16 * D], BF16, isOutput=False)
    wo = nc.declare_dram_parameter("wo", [128, 16 * H], BF16, isOutput=False)
    cosT = nc.declare_dram_parameter("cosT", [D, S], BF16, isOutput=False)
    sinT = nc.declare_dram_parameter("sinT", [D, S], BF16, isOutput=False)
    masksl = nc.declare_dram_parameter("masksl", [128, 128], BF16, isOutput=False)
    identity = nc.declare_dram_parameter("identity", [128, 128], BF16, isOutput=False)
    out = nc.declare_dram_parameter("out", [NCH * 128, H], BF16, isOutput=True)

    with tile.TileContext(nc) as tc:
        with (
            tc.tile_pool(name="const", bufs=1) as constp,
            tc.tile_pool(name="persist", bufs=1) as persist,
            tc.tile_pool(name="xtp", bufs=2) as xtp,
            tc.tile_pool(name="ropep", bufs=3) as ropep,
            tc.tile_pool(name="ptp", bufs=3) as ptp,
            tc.tile_pool(name="attnp", bufs=2) as attnp,
            tc.tile_pool(name="smallp", bufs=24) as smallp,
            tc.tile_pool(name="agp", bufs=2) as agp,
            tc.tile_pool(name="outp", bufs=1) as outp,
            tc.tile_pool(name="dram", bufs=1, space="DRAM") as dramp,
        ):
            wq_all = persist.tile([128, 16 * HPC * D], BF16, tag="wq")
            wk_all = persist.tile([128, 16 * D], BF16, tag="wk")
            wv_all = persist.tile([128, 16 * D], BF16, tag="wv")
            cos_sb = persist.tile([128, S], BF16, tag="cos")
            sin_sb = persist.tile([128, S], BF16, tag="sin")
            msl = constp.tile([128, 128], BF16, tag="msl")
            ident = constp.tile([128, 128], BF16, tag="ident")
            wo_all = persist.tile([128, 16 * H], BF16, tag="wo")

            def wqt(t, h):
                return wq_all[:, t * HPC * D + h * D: t * HPC * D + (h + 1) * D]

            QT = [persist.tile([128, S], BF16, tag=f"qt{h}", name=f"qt{h}")
                  for h in range(HPC)]
            KT = persist.tile([128, S], BF16, tag="kt")
            V = []
            for kt in range(NKT):
                vt = persist.tile([128, D + 1], BF16, tag=f"v{kt}")
                nc.gpsimd.memset(vt[:, D:D + 1], 1.0)
                V.append(vt)

            # tiny barrier collective: warms the CC stream during the
            # input-DMA/proj phase so the first real A2A doesn't pay it
            bar_in = dramp.tile([1, 128], F32, tag="bar_in", name="bar_in")
            bar_out = dramp.tile([1, 128], F32, tag="bar_out", name="bar_out",
                                 addr_space="Shared")
            nc.gpsimd.collective_compute(
                "AllReduce", mybir.AluOpType.add,
                replica_groups=[list(range(NCORES))],
                ins=[bar_in.opt()], outs=[bar_out.opt()])

            # ---- phase 1: projections (chunked over s) + RoPE ----
            with (
                tc.tile_pool(name="projps", bufs=4, space="PSUM") as projps,
                tc.tile_pool(name="vps", bufs=2, space="PSUM") as vps,
            ):
                for sc in range(NSC):
                    off = sc * SC
                    xt_all = xtp.tile([128, 16 * SC], BF16, tag="xt",
                                      name="xt_all")
                    nc.sync.dma_start(xt_all[:], xT[sc])
                    if sc == 0:
                        # the first PE matmul's wait is coalesced to ALL
                        # sync-ring DMAs issued before it, so keep the sync
                        # prefix minimal (xt0+wq); everything else rides the
                        # scalar ring, issued (program-order) before readers
                        nc.sync.dma_start(wq_all[:], wq[:])
                        nc.sync.dma_start(wk_all[:], wk[:])
                        nc.sync.dma_start(wv_all[:], wv[:])
                        nc.scalar.dma_start(cos_sb[:], cosT[:])
                        nc.scalar.dma_start(sin_sb[:], sinT[:])
                        nc.scalar.dma_start(msl[:], masksl[:])
                        nc.scalar.dma_start(ident[:], identity[:])

                    def xts(t):
                        return xt_all[:, t * SC:(t + 1) * SC]

                    # q/k projections -> transposed layout [d, s]
                    def project_rope(lhs_of_t, dst):
                        ps = projps.tile([128, SC], F32, tag="proj", name="ps")
                        for t in range(16):
                            nc.tensor.matmul(ps[:], lhs_of_t(t), xts(t),
                                             start=(t == 0), stop=(t == 15))
                        raw = ropep.tile([128, SC], BF16, tag="raw", name="raw")
                        nc.vector.tensor_copy(raw[:], ps[:])
                        sw = ropep.tile([128, SC], BF16, tag="sw", name="sw")
                        nc.sync.dma_start(sw[0:64, :], raw[64:128, :])
                        nc.sync.dma_start(sw[64:128, :], raw[0:64, :])
                        t1 = ropep.tile([128, SC], BF16, tag="t1", name="t1")
                        nc.vector.tensor_tensor(t1[:], raw[:],
                                                cos_sb[:, off:off + SC],
                                                mybir.AluOpType.mult)
                        t2 = ropep.tile([128, SC], BF16, tag="t2", name="t2")
                        nc.vector.tensor_tensor(t2[:], sw[:],
                                                sin_sb[:, off:off + SC],
                                                mybir.AluOpType.mult)
                        nc.vector.tensor_tensor(dst[:, off:off + SC], t1[:],
                                                t2[:], mybir.AluOpType.add)

                    for h in range(HPC):
                        project_rope(lambda t, h=h: wqt(t, h), QT[h])
                    project_rope(lambda t: wk_all[:, t * D:(t + 1) * D], KT)

                    # v projection (natural [s, d] layout)
                    for st in range(SC // 128):
                        v_ps = vps.tile([128, 128], F32, tag="vps", name="v_ps")
                        for t in range(16):
                            nc.tensor.matmul(
                                v_ps[:], xt_all[:, t * SC + st * 128:
                                                t * SC + (st + 1) * 128],
                                wv_all[:, t * D:(t + 1) * D],
                                start=(t == 0), stop=(t == 15))
                        kti = sc * (SC // 128) + st
                        nc.vector.tensor_copy(V[kti][:, 0:D], v_ps[:])

            # wo (8.4MB) deliberately loads during attention, not proj:
            # the proj phase is nearly DMA-bound (xt stream), attention
            # DMA is light, and wo is first read at the qs=5 o_proj fill
            nc.scalar.dma_start(wo_all[:], wo[:])

            # ---- phases 2+3 interleaved: attention, AG, o_proj ----
            with (
                tc.tile_pool(name="stps", bufs=2, space="PSUM") as stps,
                tc.tile_pool(name="attps", bufs=1, space="PSUM") as attps,
                tc.tile_pool(name="ops", bufs=1, space="PSUM") as opsp,
            ):
                attnT_cur = [None, None]
                bounces = []
                fillers = []        # pending o_proj closures (PE work units)
                hold = [0]          # units to keep queued for the tail gap

                def pump(n):
                    for _ in range(n):
                        if len(fillers) <= hold[0]:
                            return
                        fillers.pop(0)()

                def attention_pair(qs):
                    """Both heads together: per kt one paired score psum
                    [h0|h1] (2 banks), ONE 2N-wide exp, then PVs for both
                    heads into att accumulators packed 3-per-bank (memset +
                    start=False accumulation, since start=True would zero a
                    whole shared bank)."""
                    q_off = qs * QS
                    banks = [attps.tile([128, 512], F32, tag=f"ab{b}",
                                        name=f"ab{b}") for b in range(3)]
                    for b in banks:
                        nc.vector.memset(b[:], 0.0)

                    def att_ap(h, qsub):
                        i = h * 4 + qsub
                        off = (i % 3) * 160
                        return banks[i // 3][:, off:off + D + 1]

                    def norm_one(h, qsub):
                        a = att_ap(h, qsub)
                        recip = smallp.tile([128, 1], F32, tag="recip",
                                            name="recip")
                        nc.vector.reciprocal(recip[:], a[:, D:D + 1])
                        attn_n = smallp.tile([128, 128], BF16, tag="attn_n",
                                             name="attn_n")
                        nc.vector.tensor_scalar(attn_n[:], a[:, 0:D],
                                                recip[:], None,
                                                mybir.AluOpType.mult)
                        # XBAR transpose DMA ([q,j] -> [j,q]) replaces the
                        # PE transpose + DVE copy
                        col = (qs % 2) * QS + qsub * 128
                        nc.sync.dma_start_transpose(
                            attnT_cur[h][:, col:col + 128], attn_n[:])
                        pump(2)

                    def _attv(kt, j, q_lo, pt):
                        for h in range(HPC):
                            for qsub in range(max(j, 0), 4):
                                stop = (kt == 4 * qs + qsub)
                                nc.tensor.matmul(
                                    att_ap(h, qsub),
                                    pt[:, h * QS + qsub * 128 - q_lo:
                                          h * QS + qsub * 128 - q_lo + 128],
                                    V[kt][:],
                                    start=False, stop=stop,
                                    skip_group_check=True)
                                if stop:
                                    norm_one(h, qsub)

                    nkt = 4 * qs + 4
                    pend = None  # (kt, j, q_lo, pt)
                    for kt in range(nkt):
                        j = kt - 4 * qs
                        q_lo = 128 * j if j > 0 else 0
                        N = QS - q_lo
                        sp = stps.tile([128, 2 * QS], F32, tag="st",
                                       name="st_pair")
                        for h in range(HPC):
                            nc.tensor.matmul(
                                sp[:, h * QS:h * QS + N],
                                KT[:, kt * 128:(kt + 1) * 128],
                                QT[h][:, q_off + q_lo:q_off + QS],
                                start=True, stop=(j < 0),
                                skip_group_check=True)
                            if j >= 0:
                                # additive causal mask on the diagonal block
                                nc.tensor.matmul(
                                    sp[:, h * QS:h * QS + 128], ident[:],
                                    msl[:], start=False, stop=True,
                                    skip_group_check=True)
                        pt = ptp.tile([128, 2 * QS], BF16, tag="pt", name="pt")
                        src = sp[:].rearrange("p (h c) -> p h c", h=2)[:, :, 0:N]
                        dst = pt[:].rearrange("p (h c) -> p h c", h=2)[:, :, 0:N]
                        nc.scalar.activation(dst, src,
                                             mybir.ActivationFunctionType.Exp,
                                             scale=SCALE)
                        pump(2)
                        if pend is not None:
                            _attv(*pend)
                        pend = (kt, j, q_lo, pt)
                    pump(2)
                    _attv(*pend)

                def emit_a2a(ci, bounce):
                    ex = dramp.tile([NCORES * 2 * 128, 128], BF16,
                                    tag=f"a2a{ci}", name=f"a2a{ci}")
                    nc.gpsimd.collective_compute(
                        "AllToAll", mybir.AluOpType.bypass,
                        replica_groups=[list(range(NCORES))],
                        ins=[bounce.opt()], outs=[ex.opt()])
                    bounces.append(ex)

                def enqueue_oproj(ci):
                    """Queue chunk ci's o_proj as filler closures."""
                    ex = bounces[ci]
                    ag_all = agp.tile([128, 16 * 128], BF16, tag="ag",
                                      name="ag_all")
                    nc.sync.dma_start(
                        ag_all[:].rearrange("p (t s) -> p t s", t=16),
                        ex[:].rearrange("(t p) s -> p t s", p=128))
                    o_sb = outp.tile([128, H], BF16, tag="osb", name="o_sb")
                    state = {}

                    def mk_mm(ocg, jt):
                        def run():
                            if jt == 0:
                                state[ocg] = opsp.tile([128, 512], F32,
                                                       tag="ops", name="o_ps")
                            nc.tensor.matmul(
                                state[ocg][:],
                                ag_all[:, jt * 128:(jt + 1) * 128],
                                wo_all[:, jt * H + ocg * 512:
                                       jt * H + (ocg + 1) * 512],
                                start=(jt == 0), stop=(jt == 15))
                        return run

                    def mk_copy(ocg):
                        def run():
                            nc.vector.tensor_copy(
                                o_sb[:, ocg * 512:(ocg + 1) * 512],
                                state[ocg][:])
                        return run

                    def mk_store(ocg):
                        def run():
                            nc.sync.dma_start(
                                out[ci * 128:(ci + 1) * 128,
                                    ocg * 512:(ocg + 1) * 512],
                                o_sb[:, ocg * 512:(ocg + 1) * 512])
                        return run

                    for ocg in range(H // 512):
                        for jt in range(16):
                            fillers.append(mk_mm(ocg, jt))
                        fillers.append(mk_copy(ocg))
                        fillers.append(mk_store(ocg))

                bounce_cur = None
                for qs in range(NQS):
                    ci = qs // 2
                    if qs % 2 == 0:
                        bounce_cur = dramp.tile([NCORES * 2 * 128, 128], BF16,
                                                tag=f"bn{ci}", name=f"bn{ci}")
                        for head in range(HPC):
                            attnT_cur[head] = attnp.tile(
                                [128, CHUNK], BF16, tag=f"attnT{head}",
                                name=f"attnT{head}_{qs}")
                    # consume A2As late: the first A2A absorbs the one-time
                    # CC-stream setup + core launch skew, and its completion
                    # time varies, so chunks 0+1 only enter at qs=6
                    if qs == 6:
                        enqueue_oproj(0)
                        enqueue_oproj(1)
                    elif qs == 7:
                        enqueue_oproj(2)
                        hold[0] = 45   # keep PE work for the A2A(3) gap
                    attention_pair(qs)
                    if qs % 2 == 1:
                        # scatter this chunk's attnT into the A2A bounce:
                        # dest core d gets s cols d*128..(d+1)*128 of the chunk
                        bv = bounce_cur[:].rearrange(
                            "(d h j) s -> h j d s", d=NCORES, h=HPC)
                        for head in range(HPC):
                            nc.sync.dma_start(
                                bv[head],
                                attnT_cur[head][:].rearrange(
                                    "j (d s) -> j d s", d=NCORES))
                        emit_a2a(ci, bounce_cur)
                # tail: the held-back chunk-2 units cover the A2A(3) wait
                hold[0] = 0
                pump(len(fillers))
                enqueue_oproj(3)
                pump(len(fillers))

    nc.compile()
    return nc


def _get_nc():
    if "nc" not in _cached:
        _cached["nc"] = _build()
    return _cached["nc"]


def _prep_inputs(hidden_states, Wq, Wk, Wv, Wo, position_ids):
    x = np.asarray(hidden_states, dtype=np.float32).reshape(S, H)
    xT = np.ascontiguousarray(
        x.T.reshape(16, 128, NSC, SC).transpose(2, 1, 0, 3)
        .reshape(NSC, 128, 16 * SC)).astype(NPBF16)

    def wshuf(W):
        n = W.shape[1]
        return np.ascontiguousarray(
            W.reshape(16, 128, n).transpose(1, 0, 2).reshape(128, 16 * n)
        ).astype(NPBF16)
    Wq = np.asarray(Wq, dtype=np.float32)
    Wk = np.asarray(Wk, dtype=np.float32)
    Wv = np.asarray(Wv, dtype=np.float32)
    Wo = np.asarray(Wo, dtype=np.float32)
    pos = np.asarray(position_ids).reshape(S).astype(np.float32)

    half = D // 2
    inv_freq = 1.0 / (THETA ** (np.arange(half, dtype=np.float32) * 2.0 / D))
    freqs = inv_freq[:, None] * pos[None, :]          # [64, S]
    c64 = np.cos(freqs, dtype=np.float32)
    s64 = np.sin(freqs, dtype=np.float32)
    cosT = np.vstack([c64, c64]).astype(NPBF16)       # [128, S]
    sinT = np.vstack([-s64, s64]).astype(NPBF16)      # signed for rotate-half
    masksl = (np.tril(np.ones((128, 128), dtype=np.float32), -1)
              * -30000.0).astype(NPBF16)
    Wo_bf = wshuf(Wo)
    ident = np.eye(128, dtype=np.float32).astype(NPBF16)

    in_maps = []
    for c in range(NCORES):
        kvh = c // 2
        in_maps.append({
            "xT": xT,
            "wq": wshuf(Wq[:, c * HPC * D:(c + 1) * HPC * D]),
            "wk": wshuf(Wk[:, kvh * D:(kvh + 1) * D]),
            "wv": wshuf(Wv[:, kvh * D:(kvh + 1) * D]),
            "wo": Wo_bf,
            "cosT": cosT,
            "sinT": sinT,
            "masksl": masksl,
            "identity": ident,
        })
    return in_maps


def _run(inputs, trace=False):
    nc = _get_nc()
    in_maps = _prep_inputs(**inputs)
    res = run_bass_kernel_spmd(nc, in_maps, list(range(NCORES)), trace=trace)
    full = np.empty((S, H), dtype=np.float32)
    for c in range(NCORES):
        shard = np.asarray(res.results[c]["out"], dtype=np.float32)
        for i in range(NCH):
            full[i * CHUNK + c * 128: i * CHUNK + (c + 1) * 128, :] = \
                shard[i * 128:(i + 1) * 128, :]
    return full.reshape(B, S, H), res


def kernel(**inputs):
    full, _ = _run(inputs, trace=False)
    return full
